# revision 26
# baseline (speedup 1.0000x reference)
"""Trainium2 kernel for nn_Loss_HF_86079734546730.

Strategy (8 NeuronCores, SPMD, no collectives):
  - Shard the two [64,3,512,512] inputs spatially over H: core k gets raw
    rows [64k, 64k+64) => shard [64, 3, 64, 512] per tensor (48 MiB/core).
  - DMA: per (t, c): bt=0 loads 4 MiB (32 batches x one channel), bt=1
    loads as two 2 MiB h-halves (16 KiB descriptors) so the final
    DVE/band/gram chain after the last input byte is short. SBUF layout
    [128 = (b32 x qq4), 8192 = (16 h-rows x 512 w)] fp32.
  - Vertical Haar (DVE): vs/vd = x[h even] +/- x[h odd] per h-half tile
    [128, 2048] bf16 (bufs=4: fine-grained release so the band matmuls
    unblock buffer reuse early and the DMA stream never stalls), w-parity
    deinterleaved on write. vd is issued before vs for bt=1 to match PE
    order (hl/hh matmuls first) at the tile end.
  - Band build (PE): per (tile, wbc, half): 16 bf16 matmuls; stationary
    = contiguous vs/vd w-parity slice [128, 128wb] (FWL-eligible), moving
    = +/-0.5*PI permutation blocks. The horizontal Haar pass rides PSUM
    accumulation; hl and hh share one N=256 moving. PI reorders psum
    cols to (qq, b) so copies have contiguous inner runs.
  - PSUM->SBUF: 3 copies per (tile, wbc, half) cast bf16 into the band
    buffer [128 wb, 6176 cols], column g = hb*193 + colIdx, hb = half*16
    + rb4*4 + qq, colIdx = bt*96 + band*32 + b; hb-major so the Gram's
    operands are contiguous. Split scalar (lh, hh) / vector (hl) so the
    psum-tile recycle latency stays under the matmul group time.
  - Gram (PE): per (t, c): contract the band buffer over spatial into
    two PSUM accumulators pg_a (wbc=0) / pg_b (wbc=1), each holding the
    [128,193] trim plus the [65,65] corner in one bank. Gram matmul
    groups trail band production per (wbc, half) during bt=1, so after
    the last input byte only the final half's gram remains. start=True
    only on the bank's very first matmul (start clears has_written for
    the whole bank). Host sums the a/b parts.
  - Host (float64): sum partial Grams over cores, rebuild per-(b,c,band)
    mean/std, expand the normalized-feature Gram algebraically,
    cosine-sim, softmax, KL.
"""

import numpy as np

B, C, H, W = 64, 3, 512, 512
NCORES = 8
HSH = H // NCORES          # 64 raw rows per core
EPS_STD = 1e-5
EPS_COS = 1e-8
EPS_P = 1e-8

BPT = 32                   # batches per raw tile
NT = B // BPT              # 2 raw tiles per (t, c)
NBCOL = 6176               # (192 band cols + 1 ones col) x 32 hb

_CACHE = {}


def _make_w():
    """[128, 768] fp32 moving operands.

    cols 0:128   P = +0.5*PI (lh from vs_e); PI: partition p = b*4+qq ->
                 psum col qq*32 + b (contiguous inner runs for copies)
    cols 128:256 M = -0.5*PI (lh from vs_o)
    cols 256:512 w2e: vd_e -> (hl|hh), psum col qq*64 + band2*32 + b
                 (+0.5 hl, -0.5 hh) — band2 INSIDE qq so hl+hh copy
                 back as a single 4-d AP
    cols 512:768 w2o: vd_o -> (hl|hh), +0.5 for both
    """
    w = np.zeros((128, 768), np.float32)
    for b in range(BPT):
        for qq in range(4):
            p = b * 4 + qq
            w[p, qq * BPT + b] = 0.5                  # P
            w[p, 128 + qq * BPT + b] = -0.5           # M
            w[p, 256 + qq * 64 + b] = 0.5             # w2e hl
            w[p, 256 + qq * 64 + 32 + b] = -0.5       # w2e hh
            w[p, 512 + qq * 64 + b] = 0.5             # w2o hl
            w[p, 512 + qq * 64 + 32 + b] = 0.5        # w2o hh
    return w


def _col_batch():
    """band-buffer column g = colIdx*32 + hb; colIdx = bt*96 + band*32 + b
    -> batch index bt*32 + b (band order lh, hl, hh; irrelevant to host)."""
    col_batch = np.zeros(192, np.int64)
    for bt in range(NT):
        for band in range(3):
            for b in range(BPT):
                col_batch[bt * 96 + band * 32 + b] = bt * BPT + b
    return col_batch


def _build_nc():
    import concourse.mybir as mybir
    import concourse.tile as tile
    from concourse import bacc

    f32 = mybir.dt.float32
    bf16 = mybir.dt.bfloat16

    nc = bacc.Bacc()
    za = nc.declare_dram_parameter("za", [B, C, HSH, W], f32, isOutput=False)
    zs = nc.declare_dram_parameter("zs", [B, C, HSH, W], f32, isOutput=False)
    wmat = nc.declare_dram_parameter("wmat", [128, 768], bf16, isOutput=False)
    # a/b = per-wbc partial grams; host slices the [128,193] trim and
    # the [65,65] corner out of each [128,258] block and sums a+b.
    gg = nc.declare_dram_parameter("GG", [2, C, 2, 128, 258], f32, isOutput=True)
    zz = [za, zs]

    with tile.TileContext(nc) as tc:
        with (
            tc.tile_pool(name="wconst", bufs=1) as w_pool,
            tc.tile_pool(name="raw", bufs=2) as raw_pool,
            tc.tile_pool(name="vsd", bufs=4) as vsd_pool,
            tc.tile_pool(name="bands", bufs=2) as band_pool,
            tc.tile_pool(name="stage", bufs=2) as stage_pool,
            tc.tile_pool(name="pband", bufs=2, space="PSUM") as pb_pool,
            tc.tile_pool(name="pgram", bufs=2, space="PSUM") as pg_pool,
        ):
            # first raw-tile DMA issues before anything else on gpsimd so
            # input packets start flowing as early as possible
            raw_first = raw_pool.tile([128, 8192], f32, tag="raw")
            nc.gpsimd.dma_start(
                raw_first[:],
                zz[0][0:BPT, 0].rearrange("b h w -> b (h w)"),
            )

            w_t = w_pool.tile([128, 768], bf16, tag="wmat")
            nc.gpsimd.dma_start(w_t[:], wmat[:])
            wp = w_t[:, 0:128]     # +0.5*I
            wm = w_t[:, 128:256]   # -0.5*I
            w2e = w_t[:, 256:512]  # vd_e -> (hl|hh), band2-inside-qq psum cols
            w2o = w_t[:, 512:768]  # vd_o -> (hl|hh)

            def dve_chunk(raw, col0, ncol, vd_first, eng=None):
                """Vertical Haar for raw cols [col0, col0+ncol): returns
                (vsc, vdc) [128, ncol//2] bf16 chunk tiles. eng picks the
                engine (vector by default; gpsimd offload for chunks where
                the DVE is the backlog)."""
                eng = eng or nc.vector
                nrb = ncol // 1024
                rvh = raw[:, col0 : col0 + ncol].rearrange(
                    "p (rb hpar wb wpar) -> p hpar rb wpar wb",
                    rb=nrb, hpar=2, wb=256, wpar=2,
                )
                vsc = vsd_pool.tile([128, ncol // 2], bf16, tag="vs")
                vdc = vsd_pool.tile([128, ncol // 2], bf16, tag="vd")
                vsv = vsc[:].rearrange(
                    "p (rb wpar wb) -> p rb wpar wb", rb=nrb, wpar=2
                )
                vdv = vdc[:].rearrange(
                    "p (rb wpar wb) -> p rb wpar wb", rb=nrb, wpar=2
                )
                if vd_first:
                    eng.tensor_sub(vdv, rvh[:, 0], rvh[:, 1])
                    eng.tensor_add(vsv, rvh[:, 0], rvh[:, 1])
                else:
                    eng.tensor_add(vsv, rvh[:, 0], rvh[:, 1])
                    eng.tensor_sub(vdv, rvh[:, 0], rvh[:, 1])
                return vsc, vdc

            def band_mms(pb, vsc, vdc, wbc, vd_first, rb4s=range(4), rb4_0=0):
                """Band matmuls for rb4 in rb4s of one (wbc, half) into
                psum pb from chunk tiles vsc/vdc (cols local from rb4_0)."""
                def lh(rb4):
                    rl = rb4 - rb4_0
                    sve = vsc[:, rl * 512 + 128 * wbc :][:, :128]
                    svo = vsc[:, rl * 512 + 256 + 128 * wbc :][:, :128]
                    o = pb[:, rb4 * 128 : rb4 * 128 + 128]
                    nc.tensor.matmul(o, sve, wp, start=True, stop=False)
                    nc.tensor.matmul(o, svo, wm, start=False, stop=True)

                def hlhh(rb4):
                    rl = rb4 - rb4_0
                    sde = vdc[:, rl * 512 + 128 * wbc :][:, :128]
                    sdo = vdc[:, rl * 512 + 256 + 128 * wbc :][:, :128]
                    o = pb[:, 512 + rb4 * 256 : 512 + rb4 * 256 + 256]
                    nc.tensor.matmul(o, sde, w2e, start=True, stop=False)
                    nc.tensor.matmul(o, sdo, w2o, start=False, stop=True)

                if vd_first:
                    for rb4 in rb4s:
                        hlhh(rb4)
                    for rb4 in rb4s:
                        lh(rb4)
                else:
                    for rb4 in rb4s:
                        lh(rb4)
                        hlhh(rb4)

            def band_copies(pb, bb, bt, half, split=False, rsl=slice(0, 4)):
                """psum (rb4, qq, b) -> bb g = hb*193 + colIdx,
                hb = half*16 + rb4*4 + qq, colIdx = bt*96 + band*32 + b.
                All on scalar in steady state (vector CAST is no faster
                and the DVE has no slack); split hl onto vector only in
                the tail where the DVE is otherwise idle."""
                src0 = pb[:, 0:512].rearrange(
                    "p (rb4 qq b) -> p rb4 qq b", rb4=4, qq=4, b=BPT
                )[:, rsl]
                src12 = pb[:, 512:1536].rearrange(
                    "p (rb4 qq x) -> p rb4 qq x", rb4=4, qq=4, x=64
                )[:, rsl]
                bbv = bb[:].rearrange(
                    "p (h2 rb4 qq col) -> p h2 rb4 qq col",
                    h2=2, rb4=4, qq=4, col=193,
                )
                dst0 = bbv[:, half, rsl, :, bt * 96 : bt * 96 + 32]
                dst12 = bbv[:, half, rsl, :, bt * 96 + 32 : bt * 96 + 96]
                nc.scalar.activation(
                    dst0, src0, mybir.ActivationFunctionType.Copy
                )
                if split:
                    nc.vector.tensor_copy(dst12, src12)
                else:
                    nc.scalar.activation(
                        dst12, src12, mybir.ActivationFunctionType.Copy
                    )

            def gram_mms(pg, bbf, hbs, first, last):
                """Gram matmuls (trim [128,193] + corner [65,65]) over the
                hb list into psum pg [128, 258]. Both regions share one
                bank: only the bank's very first matmul may carry
                start=True (start clears has_written bank-wide); the
                corner group's first write relies on overwrite-where-
                unset semantics."""
                for i, hb in enumerate(hbs):
                    lastmm = last and i == len(hbs) - 1
                    nc.tensor.matmul(
                        pg[:, 0:193],
                        bbf[:, hb * 193 : hb * 193 + 128],
                        bbf[:, hb * 193 : hb * 193 + 193],
                        start=(first and i == 0),
                        stop=lastmm,
                    )
                    nc.tensor.matmul(
                        pg[:65, 193:258],
                        bbf[:, hb * 193 + 128 : hb * 193 + 193],
                        bbf[:, hb * 193 + 128 : hb * 193 + 193],
                        start=False,
                        stop=lastmm,
                    )

            for c in range(C):
                bufs = {}
                for t in range(2):
                    for wbc in range(2):
                        bb = band_pool.tile([128, NBCOL], bf16, tag=f"bb{t}{wbc}")
                        bbh = bb[:].rearrange("p (hb col) -> p hb col", col=193)
                        nc.gpsimd.memset(bbh[:, :, 192], 1.0)
                        bufs[(t, wbc)] = bb

                for t in range(2):
                    tail = c == C - 1 and t == 1
                    # ---- bt = 0: full 4 MiB tile ----
                    if c == 0 and t == 0:
                        raw = raw_first
                    else:
                        raw = raw_pool.tile([128, 8192], f32, tag="raw")
                        nc.gpsimd.dma_start(
                            raw[:],
                            zz[t][0:BPT, c].rearrange("b h w -> b (h w)"),
                        )
                    for hf in range(2):
                        vsh, vdh = dve_chunk(
                            raw, hf * 4096, 4096, vd_first=False,
                            eng=nc.gpsimd if hf == 0 else None,
                        )
                        for wbc in range(2):
                            pb = pb_pool.tile([128, 1536], f32, tag="pband")
                            band_mms(pb, vsh, vdh, wbc, vd_first=False)
                            band_copies(pb, bufs[(t, wbc)], 0, hf)

                    # ---- bt = 1: h-half DMAs (quarters at the very tail
                    # so the post-stream chain is short); gram trails ----
                    raw = raw_pool.tile([128, 8192], f32, tag="raw")
                    src_h = zz[t][BPT:2 * BPT, c].rearrange(
                        "b (qq h2 r) w -> h2 b qq (r w)", qq=4, h2=2, r=8
                    )
                    nc.gpsimd.dma_start(raw[:, 0:4096], src_h[0])
                    if tail:
                        src_q = zz[t][BPT:2 * BPT, c].rearrange(
                            "b (qq h4 r) w -> h4 b qq (r w)", qq=4, h4=4, r=4
                        )
                        nc.gpsimd.dma_start(raw[:, 4096:6144], src_q[2])
                        nc.gpsimd.dma_start(raw[:, 6144:8192], src_q[3])
                    else:
                        nc.gpsimd.dma_start(raw[:, 4096:8192], src_h[1])
                    pg_a = pg_pool.tile([128, 258], f32, tag="pg", name="pg_a")
                    pg_b = pg_pool.tile([128, 258], f32, tag="pg", name="pg_b")
                    pgs = [pg_a, pg_b]

                    # hf = 0: half granularity
                    vsh, vdh = dve_chunk(raw, 0, 4096, vd_first=True)
                    pbs = []
                    for wbc in range(2):
                        pb = pb_pool.tile([128, 1536], f32, tag="pband")
                        band_mms(pb, vsh, vdh, wbc, vd_first=True)
                        pbs.append(pb)
                    for wbc in range(2):
                        band_copies(pbs[wbc], bufs[(t, wbc)], 1, 0)
                    for wbc in range(2):
                        gram_mms(
                            pgs[wbc], bufs[(t, wbc)][:], range(0, 16),
                            first=True, last=False,
                        )

                    # hf = 1
                    pbs = []
                    if tail:
                        # quarter granularity: DVE/bands/copies per quarter,
                        # gram per (wbc, quarter) — shortest final chain.
                        # vector copies only on the last quarter (earlier
                        # ones would head-of-line block the DVE queue).
                        for wbc in range(2):
                            pbs.append(
                                pb_pool.tile(
                                    [128, 1536], f32, tag="pband",
                                    name=f"pbq{wbc}",
                                )
                            )
                        for qt in range(2):
                            vsq, vdq = dve_chunk(
                                raw, 4096 + qt * 2048, 2048, vd_first=True
                            )
                            rb4s = range(qt * 2, qt * 2 + 2)
                            rsl = slice(qt * 2, qt * 2 + 2)
                            for wbc in range(2):
                                band_mms(
                                    pbs[wbc], vsq, vdq, wbc, vd_first=True,
                                    rb4s=rb4s, rb4_0=qt * 2,
                                )
                            for wbc in range(2):
                                band_copies(
                                    pbs[wbc], bufs[(t, wbc)], 1, 1,
                                    split=(qt == 1), rsl=rsl,
                                )
                            for wbc in range(2):
                                gram_mms(
                                    pgs[wbc], bufs[(t, wbc)][:],
                                    range(16 + qt * 8, 16 + qt * 8 + 8),
                                    first=False, last=(qt == 1),
                                )
                    else:
                        vsh, vdh = dve_chunk(raw, 4096, 4096, vd_first=True)
                        for wbc in range(2):
                            pb = pb_pool.tile([128, 1536], f32, tag="pband")
                            band_mms(pb, vsh, vdh, wbc, vd_first=True)
                            pbs.append(pb)
                        for wbc in range(2):
                            band_copies(pbs[wbc], bufs[(t, wbc)], 1, 1)
                        for wbc in range(2):
                            gram_mms(
                                pgs[wbc], bufs[(t, wbc)][:], range(16, 32),
                                first=False, last=True,
                            )

                    for wbc in range(2):
                        st = stage_pool.tile([128, 258], f32, tag=f"st{wbc}")
                        if tail and wbc == 1:
                            nc.vector.tensor_copy(st[:], pgs[wbc][:])
                        else:
                            nc.scalar.activation(
                                st[:], pgs[wbc][:],
                                mybir.ActivationFunctionType.Copy,
                            )
                        nc.sync.dma_start(gg[t, c, wbc], st[:])
    if not nc.is_finalized():
        nc.finalize()
    return nc


def _get_nc():
    if "nc" not in _CACHE:
        _CACHE["nc"] = _build_nc()
    return _CACHE["nc"]


def _in_maps(z_ada, z_sou):
    import ml_dtypes

    wm = _make_w().astype(ml_dtypes.bfloat16)
    maps = []
    for k in range(NCORES):
        sl = slice(HSH * k, HSH * (k + 1))
        maps.append(
            {
                "za": np.ascontiguousarray(z_ada[:, :, sl, :]),
                "zs": np.ascontiguousarray(z_sou[:, :, sl, :]),
                "wmat": wm,
            }
        )
    return maps


def _host_finish(g_parts):
    """g_parts: list of per-core GG [2,3,2,128,258]; cols 0:193 = trim,
    [0:65, 193:258] = corner (rows 65:128 there are garbage — unwritten
    psum partitions DMA'd as-is and ignored here)."""
    s0 = np.zeros((2, C, 128, 193), np.float64)
    s1 = np.zeros((2, C, 65, 65), np.float64)
    for g in g_parts:
        ga = np.asarray(g, np.float64)
        s0 += ga[:, :, :, :, 0:193].sum(axis=2)
        s1 += ga[:, :, 0, 0:65, 193:258] + ga[:, :, 1, 0:65, 193:258]

    col_batch = _col_batch()
    S = float(s1[0, 0, 64, 64])

    P = np.zeros((2, B, B), np.float64)
    Bm = np.zeros((192, B), np.float64)
    Bm[np.arange(192), col_batch] = 1.0
    for t in range(2):
        for c in range(C):
            full = np.zeros((193, 193), np.float64)
            full[0:128, :] = s0[t, c]
            full[128:193, 128:193] = s1[t, c]
            full[128:193, 0:128] = s0[t, c][:, 128:193].T
            M = full[:192, :192]
            Tv = full[192, :192]
            mu = Tv / S
            var = (np.diag(M) - Tv * Tv / S) / (S - 1.0)
            sig = np.sqrt(np.maximum(var, 0.0))
            alpha = 1.0 / (3.0 * (sig + EPS_STD))
            Mc = M - np.outer(mu, Tv) - np.outer(Tv, mu) + S * np.outer(mu, mu)
            Ms = (alpha[:, None] * Mc) * alpha[None, :]
            P[t] += Bm.T @ Ms @ Bm

    sims = []
    for t in range(2):
        r = np.sqrt(np.maximum(np.diag(P[t]), 0.0))
        rc = np.maximum(r, EPS_COS)
        sims.append(P[t] / np.outer(rc, rc))

    def softmax_offdiag(sim):
        m = sim.copy()
        np.fill_diagonal(m, -np.inf)
        mx = m.max(axis=1, keepdims=True)
        e = np.exp(m - mx)
        return e / e.sum(axis=1, keepdims=True)

    p_ada = softmax_offdiag(sims[0]) + EPS_P
    p_sou = softmax_offdiag(sims[1]) + EPS_P
    kl = np.sum(p_sou * (np.log(p_sou) - np.log(p_ada))) / B
    return np.float32(kl)


def kernel(z_ada, z_sou):
    from concourse.bass_utils import run_bass_kernel_spmd

    z_ada = np.asarray(z_ada, np.float32)
    z_sou = np.asarray(z_sou, np.float32)
    nc = _get_nc()
    res = run_bass_kernel_spmd(nc, _in_maps(z_ada, z_sou), list(range(NCORES)))
    g_parts = [res.results[k]["GG"] for k in range(NCORES)]
    return _host_finish(g_parts)


# revision 27
# speedup vs baseline: 1.4008x; 1.4008x over previous
"""Trainium2 kernel for nn_Loss_HF_86079734546730.

Strategy (8 NeuronCores, SPMD, no collectives):
  - Shard the two [64,3,512,512] inputs spatially over H: core k gets raw
    rows [64k, 64k+64) => shard [64, 3, 64, 512] per tensor (48 MiB/core).
  - DMA: per (t, c): bt=0 loads 4 MiB (32 batches x one channel), bt=1
    loads as two 2 MiB h-halves (16 KiB descriptors) so the final
    DVE/band/gram chain after the last input byte is short. SBUF layout
    [128 = (b32 x qq4), 8192 = (16 h-rows x 512 w)] fp32.
  - Vertical Haar (DVE): vs/vd = x[h even] +/- x[h odd] per h-half tile
    [128, 2048] bf16 (bufs=4: fine-grained release so the band matmuls
    unblock buffer reuse early and the DMA stream never stalls), w-parity
    deinterleaved on write. vd is issued before vs for bt=1 to match PE
    order (hl/hh matmuls first) at the tile end.
  - Band build (PE): per (tile, wbc, half): 16 bf16 matmuls; stationary
    = contiguous vs/vd w-parity slice [128, 128wb] (FWL-eligible), moving
    = +/-0.5*PI permutation blocks. The horizontal Haar pass rides PSUM
    accumulation; hl and hh share one N=256 moving. PI reorders psum
    cols to (qq, b) so copies have contiguous inner runs.
  - PSUM->SBUF: 3 copies per (tile, wbc, half) cast bf16 into the band
    buffer [128 wb, 6176 cols], column g = hb*193 + colIdx, hb = half*16
    + rb4*4 + qq, colIdx = bt*96 + band*32 + b; hb-major so the Gram's
    operands are contiguous. Split scalar (lh, hh) / vector (hl) so the
    psum-tile recycle latency stays under the matmul group time.
  - Gram (PE): per (t, c): contract the band buffer over spatial into
    two PSUM accumulators pg_a (wbc=0) / pg_b (wbc=1), each holding the
    [128,193] trim plus the [65,65] corner in one bank. Gram matmul
    groups trail band production per (wbc, half) during bt=1, so after
    the last input byte only the final half's gram remains. start=True
    only on the bank's very first matmul (start clears has_written for
    the whole bank). Host sums the a/b parts.
  - Host (float64): sum partial Grams over cores, rebuild per-(b,c,band)
    mean/std, expand the normalized-feature Gram algebraically,
    cosine-sim, softmax, KL.
"""

import numpy as np

B, C, H, W = 64, 3, 512, 512
NCORES = 8
HSH = H // NCORES          # 64 raw rows per core
EPS_STD = 1e-5
EPS_COS = 1e-8
EPS_P = 1e-8

BPT = 32                   # batches per raw tile
NT = B // BPT              # 2 raw tiles per (t, c)
NBCOL = 6176               # (192 band cols + 1 ones col) x 32 hb

_CACHE = {}


def _make_w():
    """[128, 768] fp32 moving operands.

    cols 0:128   P = +0.5*PI (lh from vs_e); PI: partition p = b*4+qq ->
                 psum col qq*32 + b (contiguous inner runs for copies)
    cols 128:256 M = -0.5*PI (lh from vs_o)
    cols 256:512 w2e: vd_e -> (hl|hh), psum col qq*64 + band2*32 + b
                 (+0.5 hl, -0.5 hh) — band2 INSIDE qq so hl+hh copy
                 back as a single 4-d AP
    cols 512:768 w2o: vd_o -> (hl|hh), +0.5 for both
    """
    w = np.zeros((128, 768), np.float32)
    for b in range(BPT):
        for qq in range(4):
            p = b * 4 + qq
            w[p, qq * BPT + b] = 0.5                  # P
            w[p, 128 + qq * BPT + b] = -0.5           # M
            w[p, 256 + qq * 64 + b] = 0.5             # w2e hl
            w[p, 256 + qq * 64 + 32 + b] = -0.5       # w2e hh
            w[p, 512 + qq * 64 + b] = 0.5             # w2o hl
            w[p, 512 + qq * 64 + 32 + b] = 0.5        # w2o hh
    return w


def _col_batch():
    """band-buffer column g = colIdx*32 + hb; colIdx = bt*96 + band*32 + b
    -> batch index bt*32 + b (band order lh, hl, hh; irrelevant to host)."""
    col_batch = np.zeros(192, np.int64)
    for bt in range(NT):
        for band in range(3):
            for b in range(BPT):
                col_batch[bt * 96 + band * 32 + b] = bt * BPT + b
    return col_batch


def _build_nc():
    import concourse.mybir as mybir
    import concourse.tile as tile
    from concourse import bacc

    f32 = mybir.dt.float32
    bf16 = mybir.dt.bfloat16

    nc = bacc.Bacc()
    za = nc.declare_dram_parameter("za", [B, C, HSH, W], f32, isOutput=False)
    zs = nc.declare_dram_parameter("zs", [B, C, HSH, W], f32, isOutput=False)
    wmat = nc.declare_dram_parameter("wmat", [128, 768], bf16, isOutput=False)
    # a/b = per-wbc partial grams; host slices the [128,193] trim and
    # the [65,65] corner out of each [128,258] block and sums a+b.
    gg = nc.declare_dram_parameter("GG", [2, C, 2, 128, 258], f32, isOutput=True)
    zz = [za, zs]

    with tile.TileContext(nc) as tc:
        with (
            tc.tile_pool(name="wconst", bufs=1) as w_pool,
            tc.tile_pool(name="raw", bufs=2) as raw_pool,
            tc.tile_pool(name="vsd", bufs=4) as vsd_pool,
            tc.tile_pool(name="bands", bufs=2) as band_pool,
            tc.tile_pool(name="stage", bufs=2) as stage_pool,
            tc.tile_pool(name="pband", bufs=2, space="PSUM") as pb_pool,
            tc.tile_pool(name="pgram", bufs=2, space="PSUM") as pg_pool,
        ):
            # first raw-tile DMA issues before anything else on gpsimd so
            # input packets start flowing as early as possible
            raw_first = raw_pool.tile([128, 8192], f32, tag="raw")
            nc.gpsimd.dma_start(
                raw_first[:],
                zz[0][0:BPT, 0].rearrange("b h w -> b (h w)"),
            )

            w_t = w_pool.tile([128, 768], bf16, tag="wmat")
            nc.gpsimd.dma_start(w_t[:], wmat[:])
            wp = w_t[:, 0:128]     # +0.5*I
            wm = w_t[:, 128:256]   # -0.5*I
            w2e = w_t[:, 256:512]  # vd_e -> (hl|hh), band2-inside-qq psum cols
            w2o = w_t[:, 512:768]  # vd_o -> (hl|hh)

            def dve_chunk(raw, col0, ncol, vd_first, eng=None):
                """Vertical Haar for raw cols [col0, col0+ncol): returns
                (vsc, vdc) [128, ncol//2] bf16 chunk tiles. eng picks the
                engine (vector by default; gpsimd offload for chunks where
                the DVE is the backlog)."""
                eng = eng or nc.vector
                nrb = ncol // 1024
                rvh = raw[:, col0 : col0 + ncol].rearrange(
                    "p (rb hpar wb wpar) -> p hpar rb wpar wb",
                    rb=nrb, hpar=2, wb=256, wpar=2,
                )
                vsc = vsd_pool.tile([128, ncol // 2], bf16, tag="vs")
                vdc = vsd_pool.tile([128, ncol // 2], bf16, tag="vd")
                vsv = vsc[:].rearrange(
                    "p (rb wpar wb) -> p rb wpar wb", rb=nrb, wpar=2
                )
                vdv = vdc[:].rearrange(
                    "p (rb wpar wb) -> p rb wpar wb", rb=nrb, wpar=2
                )
                if vd_first:
                    eng.tensor_sub(vdv, rvh[:, 0], rvh[:, 1])
                    eng.tensor_add(vsv, rvh[:, 0], rvh[:, 1])
                else:
                    eng.tensor_add(vsv, rvh[:, 0], rvh[:, 1])
                    eng.tensor_sub(vdv, rvh[:, 0], rvh[:, 1])
                return vsc, vdc

            def band_mms(pb, vsc, vdc, wbc, vd_first, rb4s=range(4), rb4_0=0):
                """Band matmuls for rb4 in rb4s of one (wbc, half) into
                psum pb from chunk tiles vsc/vdc (cols local from rb4_0)."""
                def lh(rb4):
                    rl = rb4 - rb4_0
                    sve = vsc[:, rl * 512 + 128 * wbc :][:, :128]
                    svo = vsc[:, rl * 512 + 256 + 128 * wbc :][:, :128]
                    o = pb[:, rb4 * 128 : rb4 * 128 + 128]
                    nc.tensor.matmul(o, sve, wp, start=True, stop=False)
                    nc.tensor.matmul(o, svo, wm, start=False, stop=True)

                def hlhh(rb4):
                    rl = rb4 - rb4_0
                    sde = vdc[:, rl * 512 + 128 * wbc :][:, :128]
                    sdo = vdc[:, rl * 512 + 256 + 128 * wbc :][:, :128]
                    o = pb[:, 512 + rb4 * 256 : 512 + rb4 * 256 + 256]
                    nc.tensor.matmul(o, sde, w2e, start=True, stop=False)
                    nc.tensor.matmul(o, sdo, w2o, start=False, stop=True)

                if vd_first:
                    for rb4 in rb4s:
                        hlhh(rb4)
                    for rb4 in rb4s:
                        lh(rb4)
                else:
                    for rb4 in rb4s:
                        lh(rb4)
                        hlhh(rb4)

            def band_copies(pb, bb, bt, half, split=False, rsl=slice(0, 4)):
                """psum (rb4, qq, b) -> bb g = hb*193 + colIdx,
                hb = half*16 + rb4*4 + qq, colIdx = bt*96 + band*32 + b.
                All on scalar in steady state (vector CAST is no faster
                and the DVE has no slack); split hl onto vector only in
                the tail where the DVE is otherwise idle."""
                src0 = pb[:, 0:512].rearrange(
                    "p (rb4 qq b) -> p rb4 qq b", rb4=4, qq=4, b=BPT
                )[:, rsl]
                src12 = pb[:, 512:1536].rearrange(
                    "p (rb4 qq x) -> p rb4 qq x", rb4=4, qq=4, x=64
                )[:, rsl]
                bbv = bb[:].rearrange(
                    "p (h2 rb4 qq col) -> p h2 rb4 qq col",
                    h2=2, rb4=4, qq=4, col=193,
                )
                dst0 = bbv[:, half, rsl, :, bt * 96 : bt * 96 + 32]
                dst12 = bbv[:, half, rsl, :, bt * 96 + 32 : bt * 96 + 96]
                nc.scalar.activation(
                    dst0, src0, mybir.ActivationFunctionType.Copy
                )
                if split:
                    nc.vector.tensor_copy(dst12, src12)
                else:
                    nc.scalar.activation(
                        dst12, src12, mybir.ActivationFunctionType.Copy
                    )

            def gram_mms(pg, bbf, hbs, first, last):
                """Gram matmuls (trim [128,193] + corner [65,65]) over the
                hb list into psum pg [128, 258]. Both regions share one
                bank: only the bank's very first matmul may carry
                start=True (start clears has_written bank-wide); the
                corner group's first write relies on overwrite-where-
                unset semantics."""
                for i, hb in enumerate(hbs):
                    lastmm = last and i == len(hbs) - 1
                    nc.tensor.matmul(
                        pg[:, 0:193],
                        bbf[:, hb * 193 : hb * 193 + 128],
                        bbf[:, hb * 193 : hb * 193 + 193],
                        start=(first and i == 0),
                        stop=lastmm,
                    )
                    nc.tensor.matmul(
                        pg[:65, 193:258],
                        bbf[:, hb * 193 + 128 : hb * 193 + 193],
                        bbf[:, hb * 193 + 128 : hb * 193 + 193],
                        start=False,
                        stop=lastmm,
                    )

            for c in range(C):
                bufs = {}
                for t in range(2):
                    for wbc in range(2):
                        bb = band_pool.tile([128, NBCOL], bf16, tag=f"bb{t}{wbc}")
                        bbh = bb[:].rearrange("p (hb col) -> p hb col", col=193)
                        nc.gpsimd.memset(bbh[:, :, 192], 1.0)
                        bufs[(t, wbc)] = bb

                for t in range(2):
                    tail = c == C - 1 and t == 1
                    # ---- bt = 0: full 4 MiB tile ----
                    if c == 0 and t == 0:
                        raw = raw_first
                    else:
                        raw = raw_pool.tile([128, 8192], f32, tag="raw")
                        nc.gpsimd.dma_start(
                            raw[:],
                            zz[t][0:BPT, c].rearrange("b h w -> b (h w)"),
                        )
                    for hf in range(2):
                        vsh, vdh = dve_chunk(raw, hf * 4096, 4096, vd_first=False)
                        for wbc in range(2):
                            pb = pb_pool.tile([128, 1536], f32, tag="pband")
                            band_mms(pb, vsh, vdh, wbc, vd_first=False)
                            band_copies(pb, bufs[(t, wbc)], 0, hf)

                    # ---- bt = 1: h-half DMAs (quarters at the very tail
                    # so the post-stream chain is short); gram trails ----
                    raw = raw_pool.tile([128, 8192], f32, tag="raw")
                    src_h = zz[t][BPT:2 * BPT, c].rearrange(
                        "b (qq h2 r) w -> h2 b qq (r w)", qq=4, h2=2, r=8
                    )
                    nc.gpsimd.dma_start(raw[:, 0:4096], src_h[0])
                    if tail:
                        src_q = zz[t][BPT:2 * BPT, c].rearrange(
                            "b (qq h4 r) w -> h4 b qq (r w)", qq=4, h4=4, r=4
                        )
                        nc.gpsimd.dma_start(raw[:, 4096:6144], src_q[2])
                        nc.gpsimd.dma_start(raw[:, 6144:8192], src_q[3])
                    else:
                        nc.gpsimd.dma_start(raw[:, 4096:8192], src_h[1])
                    pg_a = pg_pool.tile([128, 258], f32, tag="pg", name="pg_a")
                    pg_b = pg_pool.tile([128, 258], f32, tag="pg", name="pg_b")
                    pgs = [pg_a, pg_b]

                    # hf = 0: half granularity
                    vsh, vdh = dve_chunk(raw, 0, 4096, vd_first=True)
                    pbs = []
                    for wbc in range(2):
                        pb = pb_pool.tile([128, 1536], f32, tag="pband")
                        band_mms(pb, vsh, vdh, wbc, vd_first=True)
                        pbs.append(pb)
                    for wbc in range(2):
                        band_copies(pbs[wbc], bufs[(t, wbc)], 1, 0)
                    for wbc in range(2):
                        gram_mms(
                            pgs[wbc], bufs[(t, wbc)][:], range(0, 16),
                            first=True, last=False,
                        )

                    # hf = 1
                    pbs = []
                    if tail:
                        # quarter granularity: DVE/bands/copies per quarter,
                        # gram per (wbc, quarter) — shortest final chain.
                        # vector copies only on the last quarter (earlier
                        # ones would head-of-line block the DVE queue).
                        for wbc in range(2):
                            pbs.append(
                                pb_pool.tile(
                                    [128, 1536], f32, tag="pband",
                                    name=f"pbq{wbc}",
                                )
                            )
                        for qt in range(2):
                            vsq, vdq = dve_chunk(
                                raw, 4096 + qt * 2048, 2048, vd_first=True
                            )
                            rb4s = range(qt * 2, qt * 2 + 2)
                            rsl = slice(qt * 2, qt * 2 + 2)
                            for wbc in range(2):
                                band_mms(
                                    pbs[wbc], vsq, vdq, wbc, vd_first=True,
                                    rb4s=rb4s, rb4_0=qt * 2,
                                )
                            for wbc in range(2):
                                band_copies(
                                    pbs[wbc], bufs[(t, wbc)], 1, 1,
                                    split=(qt == 1), rsl=rsl,
                                )
                            for wbc in range(2):
                                gram_mms(
                                    pgs[wbc], bufs[(t, wbc)][:],
                                    range(16 + qt * 8, 16 + qt * 8 + 8),
                                    first=False, last=(qt == 1),
                                )
                    else:
                        vsh, vdh = dve_chunk(raw, 4096, 4096, vd_first=True)
                        for wbc in range(2):
                            pb = pb_pool.tile([128, 1536], f32, tag="pband")
                            band_mms(pb, vsh, vdh, wbc, vd_first=True)
                            pbs.append(pb)
                        for wbc in range(2):
                            band_copies(pbs[wbc], bufs[(t, wbc)], 1, 1)
                        for wbc in range(2):
                            gram_mms(
                                pgs[wbc], bufs[(t, wbc)][:], range(16, 32),
                                first=False, last=True,
                            )

                    for wbc in range(2):
                        st = stage_pool.tile([128, 258], f32, tag=f"st{wbc}")
                        if tail and wbc == 1:
                            nc.vector.tensor_copy(st[:], pgs[wbc][:])
                        else:
                            nc.scalar.activation(
                                st[:], pgs[wbc][:],
                                mybir.ActivationFunctionType.Copy,
                            )
                        nc.sync.dma_start(gg[t, c, wbc], st[:])
    if not nc.is_finalized():
        nc.finalize()
    return nc


def _get_nc():
    if "nc" not in _CACHE:
        _CACHE["nc"] = _build_nc()
    return _CACHE["nc"]


def _in_maps(z_ada, z_sou):
    import ml_dtypes

    wm = _make_w().astype(ml_dtypes.bfloat16)
    maps = []
    for k in range(NCORES):
        sl = slice(HSH * k, HSH * (k + 1))
        maps.append(
            {
                "za": np.ascontiguousarray(z_ada[:, :, sl, :]),
                "zs": np.ascontiguousarray(z_sou[:, :, sl, :]),
                "wmat": wm,
            }
        )
    return maps


def _host_finish(g_parts):
    """g_parts: list of per-core GG [2,3,2,128,258]; cols 0:193 = trim,
    [0:65, 193:258] = corner (rows 65:128 there are garbage — unwritten
    psum partitions DMA'd as-is and ignored here)."""
    s0 = np.zeros((2, C, 128, 193), np.float64)
    s1 = np.zeros((2, C, 65, 65), np.float64)
    for g in g_parts:
        ga = np.asarray(g, np.float64)
        s0 += ga[:, :, :, :, 0:193].sum(axis=2)
        s1 += ga[:, :, 0, 0:65, 193:258] + ga[:, :, 1, 0:65, 193:258]

    col_batch = _col_batch()
    S = float(s1[0, 0, 64, 64])

    P = np.zeros((2, B, B), np.float64)
    Bm = np.zeros((192, B), np.float64)
    Bm[np.arange(192), col_batch] = 1.0
    for t in range(2):
        for c in range(C):
            full = np.zeros((193, 193), np.float64)
            full[0:128, :] = s0[t, c]
            full[128:193, 128:193] = s1[t, c]
            full[128:193, 0:128] = s0[t, c][:, 128:193].T
            M = full[:192, :192]
            Tv = full[192, :192]
            mu = Tv / S
            var = (np.diag(M) - Tv * Tv / S) / (S - 1.0)
            sig = np.sqrt(np.maximum(var, 0.0))
            alpha = 1.0 / (3.0 * (sig + EPS_STD))
            Mc = M - np.outer(mu, Tv) - np.outer(Tv, mu) + S * np.outer(mu, mu)
            Ms = (alpha[:, None] * Mc) * alpha[None, :]
            P[t] += Bm.T @ Ms @ Bm

    sims = []
    for t in range(2):
        r = np.sqrt(np.maximum(np.diag(P[t]), 0.0))
        rc = np.maximum(r, EPS_COS)
        sims.append(P[t] / np.outer(rc, rc))

    def softmax_offdiag(sim):
        m = sim.copy()
        np.fill_diagonal(m, -np.inf)
        mx = m.max(axis=1, keepdims=True)
        e = np.exp(m - mx)
        return e / e.sum(axis=1, keepdims=True)

    p_ada = softmax_offdiag(sims[0]) + EPS_P
    p_sou = softmax_offdiag(sims[1]) + EPS_P
    kl = np.sum(p_sou * (np.log(p_sou) - np.log(p_ada))) / B
    return np.float32(kl)


def kernel(z_ada, z_sou):
    from concourse.bass_utils import run_bass_kernel_spmd

    z_ada = np.asarray(z_ada, np.float32)
    z_sou = np.asarray(z_sou, np.float32)
    nc = _get_nc()
    res = run_bass_kernel_spmd(nc, _in_maps(z_ada, z_sou), list(range(NCORES)))
    g_parts = [res.results[k]["GG"] for k in range(NCORES)]
    return _host_finish(g_parts)
